# revision 22
# baseline (speedup 1.0000x reference)
"""HAWQ tiny classifier on 8 TRN2 cores — pure data parallel.

Per core: batch shard of 2048 rows, fed FEATURE-MAJOR (sigT [2000, 2048],
host-transposed) so GEMM1 contracts over features with no on-device
transpose or DRAM bounce.

  q  = round(sigT * 15/max|sig|)  (f32 magic-number round on ACT, exact
        bf16 out via DVE subtract; max|sig| precomputed host-side and fed
        as a [1,1] input — a replicated scale, like the weight scales)
  a1 = W1int @ q   (bf16 PE, accumulate over 16 k-chunks, streamed with
        the tile loads; all small constants arrive in ONE packed DMA so
        the big loads start immediately)
  r  = relu(a1 + b1int)           (sum_r fused into the relu activation)
  stats: (max_r, sum_r, sumsq_r) -> ONE AllGather [1,201] -> [8,201]
  q2 = round(r * 127/gmax_r)      (exact, uses gathered global max)
  BN mu/var from the gathered r-sums (pre-quantization stats; the
  quantization perturbs mu/var by O(s2^2/12) which we correct for)
  z  = q2 @ w_eff + zbias   (BN affine + linear2 folded, 2-limb bf16)
  zr = relu(z); max zr -> AllGather #2 ([2,1] col trick, no transposes)
  out = round(zr*127/gmax_z)*s3   (pass split across ACT and DVE halves)
Output written contiguous [2, 2048] per core; host transposes/concats.
"""

import os
import sys

for p in ("/opt/trn_rl_repo", "/opt/trn_rl_repo/concourse"):
    if p not in sys.path:
        sys.path.insert(0, p)

import numpy as np
import ml_dtypes

import concourse.bass as bass
import concourse.bacc as bacc
import concourse.tile as tile
import concourse.mybir as mybir
from concourse import bass_utils
from concourse._compat import with_exitstack

F32 = mybir.dt.float32
BF16 = mybir.dt.bfloat16

BATCH, D_IN, HID, OUT = 16384, 2000, 100, 2
NCORES = 8
SHARD = BATCH // NCORES          # 2048 rows per core
NK = (D_IN + 127) // 128         # 16 feature chunks (15*128 + 80)
KCH = [128] * 15 + [80]
KP = NK * 128                    # 2048 padded feature dim (w1t zero-padded)
MAGIC = 12582912.0               # 1.5 * 2**23
BN_EPS = 1e-5

_CACHE = {}


def _build(w1s: float, w2s: float):
    nc = bacc.Bacc(
        "TRN2",
        target_bir_lowering=False,
        debug=False,
        enable_asserts=False,
        num_devices=NCORES,
    )

    sigT = nc.dram_tensor("sigT", [D_IN, SHARD], F32, kind="ExternalInput")
    w1t = nc.dram_tensor("w1t", [KP, HID], BF16, kind="ExternalInput")
    w2t = nc.dram_tensor("w2t", [HID, OUT], BF16, kind="ExternalInput")
    # packed small consts: col0=b1 col1=gamma col2=beta;
    # row0: col3=gmax col4=b2_0 col5=b2_1
    pvec = nc.dram_tensor("pvec", [HID, 8], F32, kind="ExternalInput")
    ident = nc.dram_tensor("ident", [128, 128], F32, kind="ExternalInput")
    out = nc.dram_tensor("out", [OUT, SHARD], F32, kind="ExternalOutput")

    rg = [list(range(NCORES))]

    with tile.TileContext(nc) as tc:
        _kern(tc, nc, sigT, w1t, w2t, pvec, ident, out, rg, w1s, w2s)
    nc.compile()
    return nc


@with_exitstack
def _kern(ctx, tc, nc, sigT, w1t, w2t, pvec, ident, out, rg, w1s, w2s):
    sigp = ctx.enter_context(tc.tile_pool(name="sigp", bufs=4))
    qp = ctx.enter_context(tc.tile_pool(name="qp", bufs=3))
    wp = ctx.enter_context(tc.tile_pool(name="wp", bufs=1))
    hp = ctx.enter_context(tc.tile_pool(name="hp", bufs=1))      # [HID,2048]
    sp = ctx.enter_context(tc.tile_pool(name="sp", bufs=1))      # small stats
    zp = ctx.enter_context(tc.tile_pool(name="zp", bufs=1))      # [2,2048]
    psb = ctx.enter_context(tc.tile_pool(name="psb", bufs=1, space="PSUM"))
    pss = ctx.enter_context(tc.tile_pool(name="pss", bufs=4, space="PSUM"))
    dcc = ctx.enter_context(tc.tile_pool(name="dcc", bufs=1, space="DRAM"))

    # ---- kick off the first big loads before anything else ----
    sts = []
    for k in range(2):
        st = sigp.tile([128, SHARD], F32, tag="sig")
        nc.sync.dma_start(st[:], sigT[k * 128:(k + 1) * 128, :])
        sts.append(st)

    # ---- constants / replicated params (packed; scalar-engine queue) ----
    pv = sp.tile([HID, 8], F32, tag="pvec")
    nc.scalar.dma_start(pv[:], pvec[:, :])
    b1_sb = pv[:, 0:1]
    gam_sb = pv[:, 1:2]
    bet_sb = pv[:, 2:3]
    gmax_sb = pv[0:1, 3:4]
    b2_sb = pv[0:1, 4:6]
    w2t_sb = sp.tile([HID, OUT], BF16, tag="w2t")
    nc.scalar.dma_start(w2t_sb[:], w2t[:, :])
    id_sb = sp.tile([128, 128], F32, tag="ident")
    nc.scalar.dma_start(id_sb[:], ident[:, :])
    # all 16 weight chunks in one DMA: [KP,HID] viewed as [128, NK*HID]
    wall = wp.tile([128, NK * HID], BF16, tag="wall")
    nc.scalar.dma_start(wall[:].rearrange("p (k h) -> p k h", h=HID),
                        w1t.ap().rearrange("(k p) h -> p k h", p=128))
    w1c = [wall[:, k * HID:(k + 1) * HID] for k in range(NK)]
    one1 = sp.tile([1, 1], F32, tag="one1")
    nc.vector.memset(one1[:], 1.0)
    ones8 = sp.tile([8, 1], F32, tag="ones8")
    nc.vector.memset(ones8[:], 1.0)

    # ---------- helpers ----------
    def bcast(scal, n, val, tag):
        """[n,1] f32 = val * scal (scal is [1,1]); exact broadcast."""
        r = sp.tile([n, 1], F32, tag=tag)
        nc.gpsimd.partition_broadcast(r[:], scal[:])
        if val != 1.0:
            nc.vector.tensor_scalar_mul(r[:], r[:], float(val))
        return r

    def split2(src, n, tag):
        """src [n,1] f32 -> 2 (bf16, f32) [n,1] limb pairs summing to ~src."""
        outs = []
        h0 = sp.tile([n, 1], BF16, tag=f"{tag}_h0")
        nc.vector.tensor_copy(h0[:], src[:])
        f0 = sp.tile([n, 1], F32, tag=f"{tag}_f0")
        nc.vector.tensor_copy(f0[:], h0[:])
        outs.append((h0, f0))
        rem = sp.tile([n, 1], F32, tag=f"{tag}_r")
        nc.vector.tensor_tensor(rem[:], src[:], f0[:], mybir.AluOpType.subtract)
        h1 = sp.tile([n, 1], BF16, tag=f"{tag}_h1")
        nc.vector.tensor_copy(h1[:], rem[:])
        f1 = sp.tile([n, 1], F32, tag=f"{tag}_f1")
        nc.vector.tensor_copy(f1[:], h1[:])
        outs.append((h1, f1))
        return outs

    pcm_n = [0]

    def part_collapse_max(vec, n):
        """[n,1] f32 -> [1,1] max over partitions."""
        pcm_n[0] += 1
        ps = pss.tile([1, n], F32, tag="psm")
        nc.tensor.transpose(ps[:], vec[:], id_sb[:n, :n])
        r = sp.tile([1, 1], F32, tag=f"pcm{pcm_n[0]}")
        nc.vector.reduce_max(r[:], ps[:], axis=mybir.AxisListType.X)
        return r

    # ---------- scales / biases from gmax (no collective needed) ----------
    rmax = sp.tile([1, 1], F32, tag="rmax")
    nc.vector.reciprocal(rmax[:], gmax_sb)
    qsc = bcast(rmax, 128, 15.0, "qsc")        # [128,1] = 15/gmax = 1/s1
    # b1_int = clip(round(b1 / (w1s*s1)), -2, 1)
    b1sc = bcast(rmax, HID, 15.0 / w1s, "b1sc")
    t1 = sp.tile([HID, 1], F32, tag="t1")
    nc.scalar.activation(t1[:], b1_sb, mybir.ActivationFunctionType.Copy,
                         bias=MAGIC, scale=b1sc[:])
    b1i = sp.tile([HID, 1], F32, tag="b1i")
    nc.vector.tensor_scalar(b1i[:], t1[:], MAGIC, 1.0,
                            mybir.AluOpType.subtract, mybir.AluOpType.min)
    nc.vector.tensor_scalar_max(b1i[:], b1i[:], -2.0)

    # ---- streamed: load sigT chunk -> quantize -> GEMM1 ----
    ps_a1 = psb.tile([HID, SHARD], F32, tag="big")
    for k in range(NK):
        p = KCH[k]
        if k < 2:
            st = sts[k]
        else:
            st = sigp.tile([128, SHARD], F32, tag="sig")
            nc.sync.dma_start(st[:p, :], sigT[k * 128:k * 128 + p, :])
        nc.scalar.activation(st[:p, :], st[:p, :],
                             mybir.ActivationFunctionType.Copy,
                             bias=MAGIC, scale=qsc[:p])
        qt = qp.tile([128, SHARD], BF16, tag="q")
        nc.vector.tensor_scalar_sub(qt[:p, :], st[:p, :], MAGIC)
        for n in range(SHARD // 512):
            nc.tensor.matmul(ps_a1[:, n * 512:(n + 1) * 512], w1c[k][:p, :],
                             qt[:p, n * 512:(n + 1) * 512],
                             start=(k == 0), stop=(k == NK - 1))

    # ---------- relu(+fused sum) + local stats (max, sumsq of r) ----------
    r = hp.tile([HID, SHARD], F32, tag="r")
    s1 = sp.tile([HID, 4], F32, tag="s1")
    nc.scalar.activation(r[:], ps_a1[:], mybir.ActivationFunctionType.Relu,
                         bias=b1i[:], scale=1.0, accum_out=s1[:, 1:2])
    nc.vector.reduce_max(s1[:, 0:1], r[:], axis=mybir.AxisListType.X)
    scr = hp.tile([HID, SHARD], F32, tag="scr")
    nc.scalar.activation(scr[:], r[:], mybir.ActivationFunctionType.Square,
                         accum_out=s1[:, 2:3])

    ps3 = pss.tile([4, HID], F32, tag="psm")
    nc.tensor.transpose(ps3[:], s1[:], id_sb[:HID, :HID])
    srow = sp.tile([4, HID], F32, tag="srow")
    nc.vector.tensor_copy(srow[:], ps3[:])
    lmax = sp.tile([1, 1], F32, tag="lmax")
    nc.vector.reduce_max(lmax[:], srow[0:1, :], axis=mybir.AxisListType.X)

    # ---------- ONE AllGather: [max_r | sum_r(100) | sumsq_r(100)] ----------
    din = dcc.tile([1, 201], F32, tag="di_bn")
    dout = dcc.tile([8, 201], F32, tag="do_bn")
    nc.sync.dma_start(din[0:1, 0:1], lmax[:])
    nc.sync.dma_start(din[0:1, 1:101], srow[1:2, :])
    nc.sync.dma_start(din[0:1, 101:201], srow[2:3, :])
    nc.gpsimd.collective_compute(
        "AllGather", mybir.AluOpType.bypass, replica_groups=rg,
        ins=[din.opt()], outs=[dout.opt()])
    g = sp.tile([8, 201], F32, tag="g_bn")
    nc.sync.dma_start(g[:], dout[:])
    # global max of r via a row view of the gathered column 0
    grow = sp.tile([1, 8], F32, tag="grow")
    nc.sync.dma_start(grow[:], dout[:, :].rearrange("a b -> b a")[0:1, :])
    gmaxr = sp.tile([1, 1], F32, tag="gmaxr")
    nc.vector.reduce_max(gmaxr[:], grow[:], axis=mybir.AxisListType.X)

    ps_s = pss.tile([1, 200], F32, tag="psm")
    nc.tensor.matmul(ps_s[:], ones8[:], g[:, 1:201], start=True, stop=True)
    tot = sp.tile([1, 200], F32, tag="tot")
    nc.vector.tensor_copy(tot[:], ps_s[:])
    psq = pss.tile([HID, 1], F32, tag="psm")
    nc.tensor.transpose(psq[:], tot[0:1, 0:100], one1[:])
    sumq = sp.tile([HID, 1], F32, tag="sumq")
    nc.vector.tensor_copy(sumq[:], psq[:])
    psq2 = pss.tile([HID, 1], F32, tag="psm")
    nc.tensor.transpose(psq2[:], tot[0:1, 100:200], one1[:])
    sumsq = sp.tile([HID, 1], F32, tag="sumsq")
    nc.vector.tensor_copy(sumsq[:], psq2[:])

    # ---------- q2 = round(r * 127/gmax_r)  (exact, bf16) ----------
    rrm = sp.tile([1, 1], F32, tag="rrm")
    nc.vector.reciprocal(rrm[:], gmaxr[:])
    qsc2 = bcast(rrm, HID, 127.0, "qsc2")
    nc.scalar.activation(r[:], r[:], mybir.ActivationFunctionType.Copy,
                         bias=MAGIC, scale=qsc2[:])
    q2 = hp.tile([HID, SHARD], BF16, tag="q2")
    nc.vector.tensor_scalar_sub(q2[:], r[:], MAGIC)

    # ---------- BN affine from r-stats + folded linear2 coefficients ------
    # r is in integer units of sc_r = gmax*w1s/15;  s2 = gmaxr*sc_r/127.
    inv_n = 1.0 / float(BATCH)
    muq = sp.tile([HID, 1], F32, tag="muq")
    nc.vector.tensor_scalar_mul(muq[:], sumq[:], inv_n)      # mean_r
    msq = sp.tile([HID, 1], F32, tag="msq")
    nc.vector.tensor_scalar_mul(msq[:], sumsq[:], inv_n)
    mq2 = sp.tile([HID, 1], F32, tag="mq2")
    nc.scalar.square(mq2[:], muq[:])
    varq = sp.tile([HID, 1], F32, tag="varq")
    nc.vector.tensor_tensor(varq[:], msq[:], mq2[:], mybir.AluOpType.subtract)

    scb = bcast(gmax_sb, HID, w1s / 15.0, "scb")             # [HID,1] sc_r
    scb2 = sp.tile([HID, 1], F32, tag="scb2")
    nc.scalar.square(scb2[:], scb[:])
    pm = sp.tile([1, 1], F32, tag="pm")
    nc.vector.tensor_tensor(pm[:], gmaxr[:], gmax_sb, mybir.AluOpType.mult)
    s2c = w1s / (15.0 * 127.0)
    s2b = bcast(pm, HID, s2c, "s2b")                         # [HID,1] s2
    corrb = sp.tile([HID, 1], F32, tag="corrb")
    nc.scalar.square(corrb[:], s2b[:])
    nc.vector.tensor_scalar_mul(corrb[:], corrb[:], 1.0 / 12.0)

    var = sp.tile([HID, 1], F32, tag="var")
    nc.vector.tensor_tensor(var[:], varq[:], scb2[:], mybir.AluOpType.mult)
    nc.vector.tensor_tensor(var[:], var[:], corrb[:], mybir.AluOpType.add)
    nc.vector.tensor_scalar_add(var[:], var[:], BN_EPS)
    sd = sp.tile([HID, 1], F32, tag="sd")
    nc.scalar.sqrt(sd[:], var[:])
    isd = sp.tile([HID, 1], F32, tag="isd")
    nc.vector.reciprocal(isd[:], sd[:])
    abn = sp.tile([HID, 1], F32, tag="abn")
    nc.vector.tensor_tensor(abn[:], gam_sb, isd[:], mybir.AluOpType.mult)
    mu = sp.tile([HID, 1], F32, tag="mu")
    nc.vector.tensor_tensor(mu[:], muq[:], scb[:], mybir.AluOpType.mult)
    amu = sp.tile([HID, 1], F32, tag="amu")
    nc.vector.tensor_tensor(amu[:], abn[:], mu[:], mybir.AluOpType.mult)
    cbn = sp.tile([HID, 1], F32, tag="cbn")
    nc.vector.tensor_tensor(cbn[:], bet_sb, amu[:],
                            mybir.AluOpType.subtract)
    # w_eff = w2int * (abn*s2*w2s), split into 2 exact bf16 limbs
    abns = sp.tile([HID, 1], F32, tag="abns")
    nc.vector.tensor_scalar(abns[:], abn[:], s2b[:], w2s,
                            mybir.AluOpType.mult, mybir.AluOpType.mult)
    ah = split2(abns, HID, "ah")
    weffs = []
    for j in range(2):
        wj = sp.tile([HID, OUT], BF16, tag=f"weff{j}")
        nc.vector.tensor_scalar_mul(wj[:], w2t_sb[:], ah[j][1][:])
        weffs.append(wj)
    # zbias[1,2] = w2s*(cbn @ w2int) + b2i*(w2s*s2); cbn split for exactness
    ch = split2(cbn, HID, "ch")
    ps_zb = pss.tile([1, OUT], F32, tag="psm")
    for j in range(2):
        nc.tensor.matmul(ps_zb[:], ch[j][0][:], w2t_sb[:],
                         start=(j == 0), stop=(j == 1))
    zb1 = sp.tile([1, OUT], F32, tag="zb1")
    nc.vector.tensor_scalar_mul(zb1[:], ps_zb[:], w2s)
    s2_sc = sp.tile([1, 1], F32, tag="s2sc")
    nc.vector.tensor_scalar_mul(s2_sc[:], pm[:], s2c)        # [1,1] s2
    rs2 = sp.tile([1, 1], F32, tag="rs2")
    nc.vector.reciprocal(rs2[:], s2_sc[:])
    b2sc = sp.tile([1, 1], F32, tag="b2sc")
    nc.vector.tensor_scalar_mul(b2sc[:], rs2[:], 1.0 / w2s)  # 1/(w2s*s2)
    t3 = sp.tile([1, OUT], F32, tag="t3")
    nc.scalar.activation(t3[:], b2_sb, mybir.ActivationFunctionType.Copy,
                         bias=MAGIC, scale=b2sc[:])
    b2i = sp.tile([1, OUT], F32, tag="b2i")
    nc.vector.tensor_scalar(b2i[:], t3[:], MAGIC, 1.0,
                            mybir.AluOpType.subtract, mybir.AluOpType.min)
    nc.vector.tensor_scalar_max(b2i[:], b2i[:], -2.0)
    b2is = sp.tile([1, OUT], F32, tag="b2is")
    nc.vector.tensor_scalar(b2is[:], b2i[:], s2_sc[:], w2s,
                            mybir.AluOpType.mult, mybir.AluOpType.mult)
    zbias = sp.tile([1, OUT], F32, tag="zbias")
    nc.vector.tensor_tensor(zbias[:], zb1[:], b2is[:], mybir.AluOpType.add)
    ps_zbt = pss.tile([OUT, 1], F32, tag="psm")
    nc.tensor.transpose(ps_zbt[:], zbias[:], one1[:])
    zb2 = sp.tile([OUT, 1], F32, tag="zb2")
    nc.vector.tensor_copy(zb2[:], ps_zbt[:])

    # ---------- GEMM2 (2 exact bf16 limbs) + relu + quant ----------
    ps_z = psb.tile([OUT, SHARD], F32, tag="big")
    for n in range(SHARD // 512):
        for j in range(2):
            nc.tensor.matmul(ps_z[:, n * 512:(n + 1) * 512], weffs[j][:],
                             q2[:, n * 512:(n + 1) * 512],
                             start=(j == 0), stop=(j == 1))
    zr = zp.tile([OUT, SHARD], F32, tag="zr")
    nc.scalar.activation(zr[:], ps_z[:], mybir.ActivationFunctionType.Relu,
                         bias=zb2[:], scale=1.0)
    zm1 = sp.tile([OUT, 1], F32, tag="zm1")
    nc.vector.reduce_max(zm1[:], zr[:], axis=mybir.AxisListType.X)

    # ---------- AllGather #2: per-core [2,1] maxes -> [16,1] column -------
    din2 = dcc.tile([OUT, 1], F32, tag="di_z")
    dout2 = dcc.tile([8 * OUT, 1], F32, tag="do_z")
    nc.sync.dma_start(din2[:], zm1[:])
    nc.gpsimd.collective_compute(
        "AllGather", mybir.AluOpType.bypass, replica_groups=rg,
        ins=[din2.opt()], outs=[dout2.opt()])
    g4 = sp.tile([1, 8 * OUT], F32, tag="g_z")
    nc.sync.dma_start(g4[:], dout2[:, :].rearrange("a b -> b a"))
    gmaxz = sp.tile([1, 1], F32, tag="gmaxz")
    nc.vector.reduce_max(gmaxz[:], g4[:], axis=mybir.AxisListType.X)

    rmz = sp.tile([1, 1], F32, tag="rmz")
    nc.vector.reciprocal(rmz[:], gmaxz[:])
    qsc3 = bcast(rmz, OUT, 127.0, "qsc3")          # [2,1] 127/maxz
    s3b = bcast(gmaxz, OUT, 1.0 / 127.0, "s3b")    # [2,1] s3
    nbias = sp.tile([OUT, 1], F32, tag="nbias")    # -MAGIC*s3
    nc.vector.tensor_scalar_mul(nbias[:], s3b[:], -MAGIC)
    # final pass split across engines: ACT does half A then half B's rescale,
    # DVE does half B then half A's rescale. (values >= 0 so Relu == Copy)
    H = SHARD // 2
    t5 = zp.tile([OUT, SHARD], F32, tag="t5")
    osb = zp.tile([OUT, SHARD], F32, tag="osb")
    nc.scalar.activation(t5[:, :H], zr[:, :H],
                         mybir.ActivationFunctionType.Copy,
                         bias=MAGIC, scale=qsc3[:])
    nc.vector.tensor_scalar(t5[:, H:], zr[:, H:], qsc3[:], MAGIC,
                            mybir.AluOpType.mult, mybir.AluOpType.add)
    nc.vector.tensor_scalar(osb[:, :H], t5[:, :H], MAGIC, s3b[:],
                            mybir.AluOpType.subtract, mybir.AluOpType.mult)
    nc.scalar.activation(osb[:, H:], t5[:, H:],
                         mybir.ActivationFunctionType.Relu,
                         bias=nbias[:], scale=s3b[:])
    nc.sync.dma_start(out[:, :], osb[:])


def _prep(sig, W1, b1, W2, b2, gamma, beta):
    sig = np.asarray(sig, dtype=np.float32)
    W1 = np.asarray(W1, dtype=np.float32)
    W2 = np.asarray(W2, dtype=np.float32)
    w1s = float(np.max(np.abs(W1)))
    w2s = float(np.max(np.abs(W2)))
    gmax = float(np.max(np.abs(sig)))
    w1i = np.clip(np.round(W1 / w1s), -2, 1).astype(np.float32)
    w2i = np.clip(np.round(W2 / w2s), -2, 1).astype(np.float32)
    w1t = np.zeros((KP, HID), dtype=ml_dtypes.bfloat16)
    w1t[:D_IN, :] = w1i.T.astype(ml_dtypes.bfloat16)
    w2t = np.ascontiguousarray(w2i.T).astype(ml_dtypes.bfloat16)
    pvec = np.zeros((HID, 8), dtype=np.float32)
    pvec[:, 0] = np.asarray(b1, np.float32)
    pvec[:, 1] = np.asarray(gamma, np.float32)
    pvec[:, 2] = np.asarray(beta, np.float32)
    pvec[0, 3] = gmax
    pvec[0, 4:6] = np.asarray(b2, np.float32)
    # feature-major shards: [8, 2000, 2048], one contiguous pass
    sigT8 = np.ascontiguousarray(
        sig.reshape(NCORES, SHARD, D_IN).transpose(0, 2, 1))
    com = {
        "w1t": w1t,
        "w2t": w2t,
        "pvec": pvec,
        "ident": np.eye(128, dtype=np.float32),
    }
    in_maps = []
    for c in range(NCORES):
        m = dict(com)
        m["sigT"] = sigT8[c]
        in_maps.append(m)
    return w1s, w2s, in_maps


def kernel(sig, W1, b1, W2, b2, gamma, beta):
    w1s, w2s, in_maps = _prep(sig, W1, b1, W2, b2, gamma, beta)
    key = (round(w1s, 9), round(w2s, 9))
    if key not in _CACHE:
        _CACHE[key] = _build(w1s, w2s)
    nc = _CACHE[key]
    trace = os.environ.get("BASS_TRACE") == "1"
    try:
        res = bass_utils.run_bass_kernel_spmd(
            nc, in_maps, core_ids=list(range(NCORES)), trace=trace)
    except ModuleNotFoundError:
        os.environ["BASS_NEVER_TRACE"] = "1"
        res = bass_utils.run_bass_kernel_spmd(
            nc, in_maps, core_ids=list(range(NCORES)), trace=False)
    kernel.last_results = res
    return np.concatenate(
        [np.ascontiguousarray(r["out"].T) for r in res.results], axis=0)


# revision 26
# speedup vs baseline: 1.0075x; 1.0075x over previous
"""HAWQ tiny classifier on 8 TRN2 cores — pure data parallel.

Per core: batch shard of 2048 rows, fed FEATURE-MAJOR (sigT [2000, 2048],
host-transposed) so GEMM1 contracts over features with no on-device
transpose or DRAM bounce.

  q  = round(sigT * 15/max|sig|)  (f32 magic-number round on ACT, exact
        bf16 out via DVE subtract; max|sig| precomputed host-side and fed
        as a [1,1] input — a replicated scale, like the weight scales)
  a1 = W1int @ q   (bf16 PE, accumulate over 16 k-chunks, streamed with
        the tile loads; all small constants arrive in ONE packed DMA so
        the big loads start immediately)
  r  = relu(a1 + b1int)           (sum_r fused into the relu activation)
  stats: (max_r, sum_r, sumsq_r) -> ONE AllGather [1,201] -> [8,201]
  q2 = round(r * 127/gmax_r)      (exact, uses gathered global max)
  BN mu/var from the gathered r-sums (pre-quantization stats; the
  quantization perturbs mu/var by O(s2^2/12) which we correct for)
  z  = q2 @ w_eff + zbias   (BN affine + linear2 folded, 2-limb bf16)
  zr = relu(z); max zr -> AllGather #2 ([2,1] col trick, no transposes)
  out = round(zr*127/gmax_z)*s3   (pass split across ACT and DVE halves)
Output written contiguous [2, 2048] per core; host transposes/concats.
"""

import os
import sys

for p in ("/opt/trn_rl_repo", "/opt/trn_rl_repo/concourse"):
    if p not in sys.path:
        sys.path.insert(0, p)

import numpy as np
import ml_dtypes

import concourse.bass as bass
import concourse.bacc as bacc
import concourse.tile as tile
import concourse.mybir as mybir
from concourse import bass_utils
from concourse._compat import with_exitstack

F32 = mybir.dt.float32
BF16 = mybir.dt.bfloat16

BATCH, D_IN, HID, OUT = 16384, 2000, 100, 2
NCORES = 8
SHARD = BATCH // NCORES          # 2048 rows per core
NK = (D_IN + 127) // 128         # 16 feature chunks (15*128 + 80)
KCH = [128] * 15 + [80]
KP = NK * 128                    # 2048 padded feature dim (w1t zero-padded)
MAGIC = 12582912.0               # 1.5 * 2**23
BN_EPS = 1e-5

_CACHE = {}


def _build(w1s: float, w2s: float):
    nc = bacc.Bacc(
        "TRN2",
        target_bir_lowering=False,
        debug=False,
        enable_asserts=False,
        num_devices=NCORES,
    )

    sigT = nc.dram_tensor("sigT", [D_IN, SHARD], F32, kind="ExternalInput")
    w1t = nc.dram_tensor("w1t", [KP, HID], BF16, kind="ExternalInput")
    w2t = nc.dram_tensor("w2t", [HID, OUT], BF16, kind="ExternalInput")
    # packed small consts: col0=b1 col1=gamma col2=beta;
    # row0: col3=gmax col4=b2_0 col5=b2_1
    pvec = nc.dram_tensor("pvec", [HID, 8], F32, kind="ExternalInput")
    ident = nc.dram_tensor("ident", [128, 128], F32, kind="ExternalInput")
    out = nc.dram_tensor("out", [OUT, SHARD], F32, kind="ExternalOutput")

    rg = [list(range(NCORES))]

    with tile.TileContext(nc) as tc:
        _kern(tc, nc, sigT, w1t, w2t, pvec, ident, out, rg, w1s, w2s)
    nc.compile()
    return nc


@with_exitstack
def _kern(ctx, tc, nc, sigT, w1t, w2t, pvec, ident, out, rg, w1s, w2s):
    sigp = ctx.enter_context(tc.tile_pool(name="sigp", bufs=5))
    qp = ctx.enter_context(tc.tile_pool(name="qp", bufs=3))
    wp = ctx.enter_context(tc.tile_pool(name="wp", bufs=1))
    hp = ctx.enter_context(tc.tile_pool(name="hp", bufs=1))      # [HID,2048]
    sp = ctx.enter_context(tc.tile_pool(name="sp", bufs=1))      # small stats
    zp = ctx.enter_context(tc.tile_pool(name="zp", bufs=1))      # [2,2048]
    psb = ctx.enter_context(tc.tile_pool(name="psb", bufs=1, space="PSUM"))
    pss = ctx.enter_context(tc.tile_pool(name="pss", bufs=4, space="PSUM"))
    dcc = ctx.enter_context(tc.tile_pool(name="dcc", bufs=1, space="DRAM"))

    # ---- kick off the first big loads before anything else ----
    sts = []
    for k in range(2):
        st = sigp.tile([128, SHARD], F32, tag="sig")
        nc.sync.dma_start(st[:], sigT[k * 128:(k + 1) * 128, :])
        sts.append(st)

    # Warm-up collective: absorbs the cc-stream entry barrier / first-op
    # setup during the load phase, so the real AllGathers below run on a
    # warm stream. Payload is a 4-byte don't-care.
    dwu = dcc.tile([1, 1], F32, tag="di_wu")
    dwo = dcc.tile([8, 1], F32, tag="do_wu")
    nc.gpsimd.collective_compute(
        "AllGather", mybir.AluOpType.bypass, replica_groups=rg,
        ins=[dwu.opt()], outs=[dwo.opt()])

    # ---- constants / replicated params (packed; scalar-engine queue) ----
    pv = sp.tile([HID, 8], F32, tag="pvec")
    nc.scalar.dma_start(pv[:], pvec[:, :])
    b1_sb = pv[:, 0:1]
    gam_sb = pv[:, 1:2]
    bet_sb = pv[:, 2:3]
    gmax_sb = pv[0:1, 3:4]
    b2_sb = pv[0:1, 4:6]
    w2t_sb = sp.tile([HID, OUT], BF16, tag="w2t")
    nc.scalar.dma_start(w2t_sb[:], w2t[:, :])
    id_sb = sp.tile([128, 128], F32, tag="ident")
    nc.scalar.dma_start(id_sb[:], ident[:, :])
    # all 16 weight chunks in one DMA: [KP,HID] viewed as [128, NK*HID]
    wall = wp.tile([128, NK * HID], BF16, tag="wall")
    nc.scalar.dma_start(wall[:].rearrange("p (k h) -> p k h", h=HID),
                        w1t.ap().rearrange("(k p) h -> p k h", p=128))
    w1c = [wall[:, k * HID:(k + 1) * HID] for k in range(NK)]
    one1 = sp.tile([1, 1], F32, tag="one1")
    nc.vector.memset(one1[:], 1.0)
    ones8 = sp.tile([8, 1], F32, tag="ones8")
    nc.vector.memset(ones8[:], 1.0)

    # ---------- helpers ----------
    def bcast(scal, n, val, tag):
        """[n,1] f32 = val * scal (scal is [1,1]); exact broadcast."""
        r = sp.tile([n, 1], F32, tag=tag)
        nc.gpsimd.partition_broadcast(r[:], scal[:])
        if val != 1.0:
            nc.vector.tensor_scalar_mul(r[:], r[:], float(val))
        return r

    def split2(src, n, tag):
        """src [n,1] f32 -> 2 (bf16, f32) [n,1] limb pairs summing to ~src."""
        outs = []
        h0 = sp.tile([n, 1], BF16, tag=f"{tag}_h0")
        nc.vector.tensor_copy(h0[:], src[:])
        f0 = sp.tile([n, 1], F32, tag=f"{tag}_f0")
        nc.vector.tensor_copy(f0[:], h0[:])
        outs.append((h0, f0))
        rem = sp.tile([n, 1], F32, tag=f"{tag}_r")
        nc.vector.tensor_tensor(rem[:], src[:], f0[:], mybir.AluOpType.subtract)
        h1 = sp.tile([n, 1], BF16, tag=f"{tag}_h1")
        nc.vector.tensor_copy(h1[:], rem[:])
        f1 = sp.tile([n, 1], F32, tag=f"{tag}_f1")
        nc.vector.tensor_copy(f1[:], h1[:])
        outs.append((h1, f1))
        return outs

    pcm_n = [0]

    def part_collapse_max(vec, n):
        """[n,1] f32 -> [1,1] max over partitions."""
        pcm_n[0] += 1
        ps = pss.tile([1, n], F32, tag="psm")
        nc.tensor.transpose(ps[:], vec[:], id_sb[:n, :n])
        r = sp.tile([1, 1], F32, tag=f"pcm{pcm_n[0]}")
        nc.vector.reduce_max(r[:], ps[:], axis=mybir.AxisListType.X)
        return r

    # ---------- scales / biases from gmax (no collective needed) ----------
    rmax = sp.tile([1, 1], F32, tag="rmax")
    nc.vector.reciprocal(rmax[:], gmax_sb)
    qsc = bcast(rmax, 128, 15.0, "qsc")        # [128,1] = 15/gmax = 1/s1
    # b1_int = clip(round(b1 / (w1s*s1)), -2, 1)
    b1sc = bcast(rmax, HID, 15.0 / w1s, "b1sc")
    t1 = sp.tile([HID, 1], F32, tag="t1")
    nc.scalar.activation(t1[:], b1_sb, mybir.ActivationFunctionType.Copy,
                         bias=MAGIC, scale=b1sc[:])
    b1i = sp.tile([HID, 1], F32, tag="b1i")
    nc.vector.tensor_scalar(b1i[:], t1[:], MAGIC, 1.0,
                            mybir.AluOpType.subtract, mybir.AluOpType.min)
    nc.vector.tensor_scalar_max(b1i[:], b1i[:], -2.0)

    # ---- streamed: load sigT chunk -> quantize -> GEMM1 ----
    ps_a1 = psb.tile([HID, SHARD], F32, tag="big")
    for k in range(NK):
        p = KCH[k]
        if k < 2:
            st = sts[k]
        else:
            st = sigp.tile([128, SHARD], F32, tag="sig")
            nc.sync.dma_start(st[:p, :], sigT[k * 128:k * 128 + p, :])
        nc.scalar.activation(st[:p, :], st[:p, :],
                             mybir.ActivationFunctionType.Copy,
                             bias=MAGIC, scale=qsc[:p])
        qt = qp.tile([128, SHARD], BF16, tag="q")
        nc.vector.tensor_scalar_sub(qt[:p, :], st[:p, :], MAGIC)
        for n in range(SHARD // 512):
            nc.tensor.matmul(ps_a1[:, n * 512:(n + 1) * 512], w1c[k][:p, :],
                             qt[:p, n * 512:(n + 1) * 512],
                             start=(k == 0), stop=(k == NK - 1))

    # ---------- relu(+fused sum) + local stats (max, sumsq of r) ----------
    r = hp.tile([HID, SHARD], F32, tag="r")
    s1 = sp.tile([HID, 4], F32, tag="s1")
    nc.scalar.activation(r[:], ps_a1[:], mybir.ActivationFunctionType.Relu,
                         bias=b1i[:], scale=1.0, accum_out=s1[:, 1:2])
    nc.vector.reduce_max(s1[:, 0:1], r[:], axis=mybir.AxisListType.X)
    scr = hp.tile([HID, SHARD], F32, tag="scr")
    nc.scalar.activation(scr[:], r[:], mybir.ActivationFunctionType.Square,
                         accum_out=s1[:, 2:3])

    ps3 = pss.tile([4, HID], F32, tag="psm")
    nc.tensor.transpose(ps3[:], s1[:], id_sb[:HID, :HID])
    srow = sp.tile([4, HID], F32, tag="srow")
    nc.vector.tensor_copy(srow[:], ps3[:])
    lmax = sp.tile([1, 1], F32, tag="lmax")
    nc.vector.reduce_max(lmax[:], srow[0:1, :], axis=mybir.AxisListType.X)

    # ---------- ONE AllGather: [max_r | sum_r(100) | sumsq_r(100)] ----------
    din = dcc.tile([1, 201], F32, tag="di_bn")
    dout = dcc.tile([8, 201], F32, tag="do_bn")
    nc.sync.dma_start(din[0:1, 0:1], lmax[:])
    nc.scalar.dma_start(din[0:1, 1:101], srow[1:2, :])
    nc.sync.dma_start(din[0:1, 101:201], srow[2:3, :])
    nc.gpsimd.collective_compute(
        "AllGather", mybir.AluOpType.bypass, replica_groups=rg,
        ins=[din.opt()], outs=[dout.opt()])
    g = sp.tile([8, 201], F32, tag="g_bn")
    nc.sync.dma_start(g[:], dout[:])
    # global max of r via a row view of the gathered column 0
    grow = sp.tile([1, 8], F32, tag="grow")
    nc.sync.dma_start(grow[:], dout[:, :].rearrange("a b -> b a")[0:1, :])
    gmaxr = sp.tile([1, 1], F32, tag="gmaxr")
    nc.vector.reduce_max(gmaxr[:], grow[:], axis=mybir.AxisListType.X)

    ps_s = pss.tile([1, 200], F32, tag="psm")
    nc.tensor.matmul(ps_s[:], ones8[:], g[:, 1:201], start=True, stop=True)
    tot = sp.tile([1, 200], F32, tag="tot")
    nc.vector.tensor_copy(tot[:], ps_s[:])
    psq = pss.tile([HID, 1], F32, tag="psm")
    nc.tensor.transpose(psq[:], tot[0:1, 0:100], one1[:])
    sumq = sp.tile([HID, 1], F32, tag="sumq")
    nc.vector.tensor_copy(sumq[:], psq[:])
    psq2 = pss.tile([HID, 1], F32, tag="psm")
    nc.tensor.transpose(psq2[:], tot[0:1, 100:200], one1[:])
    sumsq = sp.tile([HID, 1], F32, tag="sumsq")
    nc.vector.tensor_copy(sumsq[:], psq2[:])

    # ---------- q2 = round(r * 127/gmax_r)  (exact, bf16) ----------
    # pass split across engines: ACT handles half A's magic-add and half B's
    # subtract; DVE the converse. Both compute scale*x+bias in f32, so the
    # halves are bit-identical to the single-engine version.
    rrm = sp.tile([1, 1], F32, tag="rrm")
    nc.vector.reciprocal(rrm[:], gmaxr[:])
    qsc2 = bcast(rrm, HID, 127.0, "qsc2")
    Hq = SHARD // 2
    q2 = hp.tile([HID, SHARD], BF16, tag="q2")
    nc.scalar.activation(r[:, :Hq], r[:, :Hq],
                         mybir.ActivationFunctionType.Copy,
                         bias=MAGIC, scale=qsc2[:])
    nc.vector.tensor_scalar(r[:, Hq:], r[:, Hq:], qsc2[:], MAGIC,
                            mybir.AluOpType.mult, mybir.AluOpType.add)
    nc.vector.tensor_scalar_sub(q2[:, :Hq], r[:, :Hq], MAGIC)
    nc.scalar.activation(q2[:, Hq:], r[:, Hq:],
                         mybir.ActivationFunctionType.Copy,
                         bias=-MAGIC, scale=1.0)

    # ---------- BN affine from r-stats + folded linear2 coefficients ------
    # r is in integer units of sc_r = gmax*w1s/15;  s2 = gmaxr*sc_r/127.
    inv_n = 1.0 / float(BATCH)
    muq = sp.tile([HID, 1], F32, tag="muq")
    nc.vector.tensor_scalar_mul(muq[:], sumq[:], inv_n)      # mean_r
    msq = sp.tile([HID, 1], F32, tag="msq")
    nc.vector.tensor_scalar_mul(msq[:], sumsq[:], inv_n)
    mq2 = sp.tile([HID, 1], F32, tag="mq2")
    nc.scalar.square(mq2[:], muq[:])
    varq = sp.tile([HID, 1], F32, tag="varq")
    nc.vector.tensor_tensor(varq[:], msq[:], mq2[:], mybir.AluOpType.subtract)

    scb = bcast(gmax_sb, HID, w1s / 15.0, "scb")             # [HID,1] sc_r
    scb2 = sp.tile([HID, 1], F32, tag="scb2")
    nc.scalar.square(scb2[:], scb[:])
    pm = sp.tile([1, 1], F32, tag="pm")
    nc.vector.tensor_tensor(pm[:], gmaxr[:], gmax_sb, mybir.AluOpType.mult)
    s2c = w1s / (15.0 * 127.0)
    s2b = bcast(pm, HID, s2c, "s2b")                         # [HID,1] s2
    corrb = sp.tile([HID, 1], F32, tag="corrb")
    nc.scalar.square(corrb[:], s2b[:])
    nc.vector.tensor_scalar_mul(corrb[:], corrb[:], 1.0 / 12.0)

    var = sp.tile([HID, 1], F32, tag="var")
    nc.vector.tensor_tensor(var[:], varq[:], scb2[:], mybir.AluOpType.mult)
    nc.vector.tensor_tensor(var[:], var[:], corrb[:], mybir.AluOpType.add)
    nc.vector.tensor_scalar_add(var[:], var[:], BN_EPS)
    sd = sp.tile([HID, 1], F32, tag="sd")
    nc.scalar.sqrt(sd[:], var[:])
    isd = sp.tile([HID, 1], F32, tag="isd")
    nc.vector.reciprocal(isd[:], sd[:])
    abn = sp.tile([HID, 1], F32, tag="abn")
    nc.vector.tensor_tensor(abn[:], gam_sb, isd[:], mybir.AluOpType.mult)
    mu = sp.tile([HID, 1], F32, tag="mu")
    nc.vector.tensor_tensor(mu[:], muq[:], scb[:], mybir.AluOpType.mult)
    amu = sp.tile([HID, 1], F32, tag="amu")
    nc.vector.tensor_tensor(amu[:], abn[:], mu[:], mybir.AluOpType.mult)
    cbn = sp.tile([HID, 1], F32, tag="cbn")
    nc.vector.tensor_tensor(cbn[:], bet_sb, amu[:],
                            mybir.AluOpType.subtract)
    # w_eff = w2int * (abn*s2*w2s), split into 2 exact bf16 limbs
    abns = sp.tile([HID, 1], F32, tag="abns")
    nc.vector.tensor_scalar(abns[:], abn[:], s2b[:], w2s,
                            mybir.AluOpType.mult, mybir.AluOpType.mult)
    ah = split2(abns, HID, "ah")
    weffs = []
    for j in range(2):
        wj = sp.tile([HID, OUT], BF16, tag=f"weff{j}")
        nc.vector.tensor_scalar_mul(wj[:], w2t_sb[:], ah[j][1][:])
        weffs.append(wj)
    # zbias[1,2] = w2s*(cbn @ w2int) + b2i*(w2s*s2); cbn split for exactness
    ch = split2(cbn, HID, "ch")
    ps_zb = pss.tile([1, OUT], F32, tag="psm")
    for j in range(2):
        nc.tensor.matmul(ps_zb[:], ch[j][0][:], w2t_sb[:],
                         start=(j == 0), stop=(j == 1))
    zb1 = sp.tile([1, OUT], F32, tag="zb1")
    nc.vector.tensor_scalar_mul(zb1[:], ps_zb[:], w2s)
    s2_sc = sp.tile([1, 1], F32, tag="s2sc")
    nc.vector.tensor_scalar_mul(s2_sc[:], pm[:], s2c)        # [1,1] s2
    rs2 = sp.tile([1, 1], F32, tag="rs2")
    nc.vector.reciprocal(rs2[:], s2_sc[:])
    b2sc = sp.tile([1, 1], F32, tag="b2sc")
    nc.vector.tensor_scalar_mul(b2sc[:], rs2[:], 1.0 / w2s)  # 1/(w2s*s2)
    t3 = sp.tile([1, OUT], F32, tag="t3")
    nc.scalar.activation(t3[:], b2_sb, mybir.ActivationFunctionType.Copy,
                         bias=MAGIC, scale=b2sc[:])
    b2i = sp.tile([1, OUT], F32, tag="b2i")
    nc.vector.tensor_scalar(b2i[:], t3[:], MAGIC, 1.0,
                            mybir.AluOpType.subtract, mybir.AluOpType.min)
    nc.vector.tensor_scalar_max(b2i[:], b2i[:], -2.0)
    b2is = sp.tile([1, OUT], F32, tag="b2is")
    nc.vector.tensor_scalar(b2is[:], b2i[:], s2_sc[:], w2s,
                            mybir.AluOpType.mult, mybir.AluOpType.mult)
    zbias = sp.tile([1, OUT], F32, tag="zbias")
    nc.vector.tensor_tensor(zbias[:], zb1[:], b2is[:], mybir.AluOpType.add)
    ps_zbt = pss.tile([OUT, 1], F32, tag="psm")
    nc.tensor.transpose(ps_zbt[:], zbias[:], one1[:])
    zb2 = sp.tile([OUT, 1], F32, tag="zb2")
    nc.vector.tensor_copy(zb2[:], ps_zbt[:])

    # ---------- GEMM2 (2 exact bf16 limbs) + relu + quant ----------
    ps_z = psb.tile([OUT, SHARD], F32, tag="big")
    for n in range(SHARD // 512):
        for j in range(2):
            nc.tensor.matmul(ps_z[:, n * 512:(n + 1) * 512], weffs[j][:],
                             q2[:, n * 512:(n + 1) * 512],
                             start=(j == 0), stop=(j == 1))
    zr = zp.tile([OUT, SHARD], F32, tag="zr")
    nc.scalar.activation(zr[:], ps_z[:], mybir.ActivationFunctionType.Relu,
                         bias=zb2[:], scale=1.0)
    zm1 = sp.tile([OUT, 1], F32, tag="zm1")
    nc.vector.reduce_max(zm1[:], zr[:], axis=mybir.AxisListType.X)

    # ---------- AllGather #2: per-core [2,1] maxes -> [16,1] column -------
    din2 = dcc.tile([OUT, 1], F32, tag="di_z")
    dout2 = dcc.tile([8 * OUT, 1], F32, tag="do_z")
    nc.sync.dma_start(din2[:], zm1[:])
    nc.gpsimd.collective_compute(
        "AllGather", mybir.AluOpType.bypass, replica_groups=rg,
        ins=[din2.opt()], outs=[dout2.opt()])
    g4 = sp.tile([1, 8 * OUT], F32, tag="g_z")
    nc.sync.dma_start(g4[:], dout2[:, :].rearrange("a b -> b a"))
    gmaxz = sp.tile([1, 1], F32, tag="gmaxz")
    nc.vector.reduce_max(gmaxz[:], g4[:], axis=mybir.AxisListType.X)

    rmz = sp.tile([1, 1], F32, tag="rmz")
    nc.vector.reciprocal(rmz[:], gmaxz[:])
    qsc3 = bcast(rmz, OUT, 127.0, "qsc3")          # [2,1] 127/maxz
    s3b = bcast(gmaxz, OUT, 1.0 / 127.0, "s3b")    # [2,1] s3
    nbias = sp.tile([OUT, 1], F32, tag="nbias")    # -MAGIC*s3
    nc.vector.tensor_scalar_mul(nbias[:], s3b[:], -MAGIC)
    # final pass split across engines: ACT does half A then half B's rescale,
    # DVE does half B then half A's rescale. (values >= 0 so Relu == Copy)
    H = SHARD // 2
    t5 = zp.tile([OUT, SHARD], F32, tag="t5")
    osb = zp.tile([OUT, SHARD], F32, tag="osb")
    nc.scalar.activation(t5[:, :H], zr[:, :H],
                         mybir.ActivationFunctionType.Copy,
                         bias=MAGIC, scale=qsc3[:])
    nc.vector.tensor_scalar(t5[:, H:], zr[:, H:], qsc3[:], MAGIC,
                            mybir.AluOpType.mult, mybir.AluOpType.add)
    nc.vector.tensor_scalar(osb[:, :H], t5[:, :H], MAGIC, s3b[:],
                            mybir.AluOpType.subtract, mybir.AluOpType.mult)
    nc.scalar.activation(osb[:, H:], t5[:, H:],
                         mybir.ActivationFunctionType.Relu,
                         bias=nbias[:], scale=s3b[:])
    nc.sync.dma_start(out[:, :], osb[:])


def _prep(sig, W1, b1, W2, b2, gamma, beta):
    sig = np.asarray(sig, dtype=np.float32)
    W1 = np.asarray(W1, dtype=np.float32)
    W2 = np.asarray(W2, dtype=np.float32)
    w1s = float(np.max(np.abs(W1)))
    w2s = float(np.max(np.abs(W2)))
    gmax = float(np.max(np.abs(sig)))
    w1i = np.clip(np.round(W1 / w1s), -2, 1).astype(np.float32)
    w2i = np.clip(np.round(W2 / w2s), -2, 1).astype(np.float32)
    w1t = np.zeros((KP, HID), dtype=ml_dtypes.bfloat16)
    w1t[:D_IN, :] = w1i.T.astype(ml_dtypes.bfloat16)
    w2t = np.ascontiguousarray(w2i.T).astype(ml_dtypes.bfloat16)
    pvec = np.zeros((HID, 8), dtype=np.float32)
    pvec[:, 0] = np.asarray(b1, np.float32)
    pvec[:, 1] = np.asarray(gamma, np.float32)
    pvec[:, 2] = np.asarray(beta, np.float32)
    pvec[0, 3] = gmax
    pvec[0, 4:6] = np.asarray(b2, np.float32)
    # feature-major shards: [8, 2000, 2048], one contiguous pass
    sigT8 = np.ascontiguousarray(
        sig.reshape(NCORES, SHARD, D_IN).transpose(0, 2, 1))
    com = {
        "w1t": w1t,
        "w2t": w2t,
        "pvec": pvec,
        "ident": np.eye(128, dtype=np.float32),
    }
    in_maps = []
    for c in range(NCORES):
        m = dict(com)
        m["sigT"] = sigT8[c]
        in_maps.append(m)
    return w1s, w2s, in_maps


def kernel(sig, W1, b1, W2, b2, gamma, beta):
    w1s, w2s, in_maps = _prep(sig, W1, b1, W2, b2, gamma, beta)
    key = (round(w1s, 9), round(w2s, 9))
    if key not in _CACHE:
        _CACHE[key] = _build(w1s, w2s)
    nc = _CACHE[key]
    trace = os.environ.get("BASS_TRACE") == "1"
    try:
        res = bass_utils.run_bass_kernel_spmd(
            nc, in_maps, core_ids=list(range(NCORES)), trace=trace)
    except ModuleNotFoundError:
        os.environ["BASS_NEVER_TRACE"] = "1"
        res = bass_utils.run_bass_kernel_spmd(
            nc, in_maps, core_ids=list(range(NCORES)), trace=False)
    kernel.last_results = res
    return np.concatenate(
        [np.ascontiguousarray(r["out"].T) for r in res.results], axis=0)


# revision 27
# speedup vs baseline: 1.0157x; 1.0081x over previous
"""HAWQ tiny classifier on 8 TRN2 cores — pure data parallel.

Per core: batch shard of 2048 rows, fed FEATURE-MAJOR (sigT [2000, 2048],
host-transposed) so GEMM1 contracts over features with no on-device
transpose or DRAM bounce.

  q  = round(sigT * 15/max|sig|)  (f32 magic-number round on ACT, exact
        bf16 out via DVE subtract; max|sig| precomputed host-side and fed
        as a [1,1] input — a replicated scale, like the weight scales)
  a1 = W1int @ q   (bf16 PE, accumulate over 16 k-chunks, streamed with
        the tile loads; all small constants arrive in ONE packed DMA so
        the big loads start immediately)
  r  = relu(a1 + b1int)           (sum_r fused into the relu activation)
  stats: (max_r, sum_r, sumsq_r) -> ONE AllGather [1,201] -> [8,201]
  q2 = round(r * 127/gmax_r)      (exact, uses gathered global max)
  BN mu/var from the gathered r-sums (pre-quantization stats; the
  quantization perturbs mu/var by O(s2^2/12) which we correct for)
  z  = q2 @ w_eff + zbias   (BN affine + linear2 folded, 2-limb bf16)
  zr = relu(z); max zr -> AllGather #2 ([2,1] col trick, no transposes)
  out = round(zr*127/gmax_z)*s3   (pass split across ACT and DVE halves)
Output written contiguous [2, 2048] per core; host transposes/concats.
"""

import os
import sys

for p in ("/opt/trn_rl_repo", "/opt/trn_rl_repo/concourse"):
    if p not in sys.path:
        sys.path.insert(0, p)

import numpy as np
import ml_dtypes

import concourse.bass as bass
import concourse.bacc as bacc
import concourse.tile as tile
import concourse.mybir as mybir
from concourse import bass_utils
from concourse._compat import with_exitstack

F32 = mybir.dt.float32
BF16 = mybir.dt.bfloat16

BATCH, D_IN, HID, OUT = 16384, 2000, 100, 2
NCORES = 8
SHARD = BATCH // NCORES          # 2048 rows per core
NK = (D_IN + 127) // 128         # 16 feature chunks (15*128 + 80)
KCH = [128] * 15 + [80]
KP = NK * 128                    # 2048 padded feature dim (w1t zero-padded)
MAGIC = 12582912.0               # 1.5 * 2**23
BN_EPS = 1e-5

_CACHE = {}


def _build(w1s: float, w2s: float):
    nc = bacc.Bacc(
        "TRN2",
        target_bir_lowering=False,
        debug=False,
        enable_asserts=False,
        num_devices=NCORES,
    )

    sigT = nc.dram_tensor("sigT", [D_IN, SHARD], F32, kind="ExternalInput")
    w1t = nc.dram_tensor("w1t", [KP, HID], BF16, kind="ExternalInput")
    w2t = nc.dram_tensor("w2t", [HID, OUT], BF16, kind="ExternalInput")
    # packed small consts: col0=b1 col1=gamma col2=beta;
    # row0: col3=gmax col4=b2_0 col5=b2_1
    pvec = nc.dram_tensor("pvec", [HID, 8], F32, kind="ExternalInput")
    ident = nc.dram_tensor("ident", [128, 128], F32, kind="ExternalInput")
    out = nc.dram_tensor("out", [OUT, SHARD], F32, kind="ExternalOutput")

    rg = [list(range(NCORES))]

    with tile.TileContext(nc) as tc:
        _kern(tc, nc, sigT, w1t, w2t, pvec, ident, out, rg, w1s, w2s)
    nc.compile()
    return nc


@with_exitstack
def _kern(ctx, tc, nc, sigT, w1t, w2t, pvec, ident, out, rg, w1s, w2s):
    sigp = ctx.enter_context(tc.tile_pool(name="sigp", bufs=5))
    qp = ctx.enter_context(tc.tile_pool(name="qp", bufs=3))
    wp = ctx.enter_context(tc.tile_pool(name="wp", bufs=1))
    hp = ctx.enter_context(tc.tile_pool(name="hp", bufs=1))      # [HID,2048]
    sp = ctx.enter_context(tc.tile_pool(name="sp", bufs=1))      # small stats
    zp = ctx.enter_context(tc.tile_pool(name="zp", bufs=1))      # [2,2048]
    psb = ctx.enter_context(tc.tile_pool(name="psb", bufs=1, space="PSUM"))
    pss = ctx.enter_context(tc.tile_pool(name="pss", bufs=4, space="PSUM"))
    dcc = ctx.enter_context(tc.tile_pool(name="dcc", bufs=1, space="DRAM"))

    # ---- kick off the first big loads before anything else ----
    sts = []
    for k in range(2):
        st = sigp.tile([128, SHARD], F32, tag="sig")
        nc.sync.dma_start(st[:], sigT[k * 128:(k + 1) * 128, :])
        sts.append(st)

    # ---- constants / replicated params (packed; scalar-engine queue) ----
    pv = sp.tile([HID, 8], F32, tag="pvec")
    nc.scalar.dma_start(pv[:], pvec[:, :])
    b1_sb = pv[:, 0:1]
    gam_sb = pv[:, 1:2]
    bet_sb = pv[:, 2:3]
    gmax_sb = pv[0:1, 3:4]
    b2_sb = pv[0:1, 4:6]
    w2t_sb = sp.tile([HID, OUT], BF16, tag="w2t")
    nc.scalar.dma_start(w2t_sb[:], w2t[:, :])
    id_sb = sp.tile([128, 128], F32, tag="ident")
    nc.scalar.dma_start(id_sb[:], ident[:, :])
    # all 16 weight chunks in one DMA: [KP,HID] viewed as [128, NK*HID]
    wall = wp.tile([128, NK * HID], BF16, tag="wall")
    nc.scalar.dma_start(wall[:].rearrange("p (k h) -> p k h", h=HID),
                        w1t.ap().rearrange("(k p) h -> p k h", p=128))
    w1c = [wall[:, k * HID:(k + 1) * HID] for k in range(NK)]
    one1 = sp.tile([1, 1], F32, tag="one1")
    nc.vector.memset(one1[:], 1.0)
    ones8 = sp.tile([8, 1], F32, tag="ones8")
    nc.vector.memset(ones8[:], 1.0)

    # ---------- helpers ----------
    def bcast(scal, n, val, tag):
        """[n,1] f32 = val * scal (scal is [1,1]); exact broadcast."""
        r = sp.tile([n, 1], F32, tag=tag)
        nc.gpsimd.partition_broadcast(r[:], scal[:])
        if val != 1.0:
            nc.vector.tensor_scalar_mul(r[:], r[:], float(val))
        return r

    def split2(src, n, tag):
        """src [n,1] f32 -> 2 (bf16, f32) [n,1] limb pairs summing to ~src."""
        outs = []
        h0 = sp.tile([n, 1], BF16, tag=f"{tag}_h0")
        nc.vector.tensor_copy(h0[:], src[:])
        f0 = sp.tile([n, 1], F32, tag=f"{tag}_f0")
        nc.vector.tensor_copy(f0[:], h0[:])
        outs.append((h0, f0))
        rem = sp.tile([n, 1], F32, tag=f"{tag}_r")
        nc.vector.tensor_tensor(rem[:], src[:], f0[:], mybir.AluOpType.subtract)
        h1 = sp.tile([n, 1], BF16, tag=f"{tag}_h1")
        nc.vector.tensor_copy(h1[:], rem[:])
        f1 = sp.tile([n, 1], F32, tag=f"{tag}_f1")
        nc.vector.tensor_copy(f1[:], h1[:])
        outs.append((h1, f1))
        return outs

    pcm_n = [0]

    def part_collapse_max(vec, n):
        """[n,1] f32 -> [1,1] max over partitions."""
        pcm_n[0] += 1
        ps = pss.tile([1, n], F32, tag="psm")
        nc.tensor.transpose(ps[:], vec[:], id_sb[:n, :n])
        r = sp.tile([1, 1], F32, tag=f"pcm{pcm_n[0]}")
        nc.vector.reduce_max(r[:], ps[:], axis=mybir.AxisListType.X)
        return r

    # ---------- scales / biases from gmax (no collective needed) ----------
    rmax = sp.tile([1, 1], F32, tag="rmax")
    nc.vector.reciprocal(rmax[:], gmax_sb)
    qsc = bcast(rmax, 128, 15.0, "qsc")        # [128,1] = 15/gmax = 1/s1
    # b1_int = clip(round(b1 / (w1s*s1)), -2, 1)
    b1sc = bcast(rmax, HID, 15.0 / w1s, "b1sc")
    t1 = sp.tile([HID, 1], F32, tag="t1")
    nc.scalar.activation(t1[:], b1_sb, mybir.ActivationFunctionType.Copy,
                         bias=MAGIC, scale=b1sc[:])
    b1i = sp.tile([HID, 1], F32, tag="b1i")
    nc.vector.tensor_scalar(b1i[:], t1[:], MAGIC, 1.0,
                            mybir.AluOpType.subtract, mybir.AluOpType.min)
    nc.vector.tensor_scalar_max(b1i[:], b1i[:], -2.0)

    # ---- streamed: load sigT chunk -> quantize -> GEMM1 ----
    ps_a1 = psb.tile([HID, SHARD], F32, tag="big")
    for k in range(NK):
        p = KCH[k]
        if k < 2:
            st = sts[k]
        else:
            st = sigp.tile([128, SHARD], F32, tag="sig")
            nc.sync.dma_start(st[:p, :], sigT[k * 128:k * 128 + p, :])
        nc.scalar.activation(st[:p, :], st[:p, :],
                             mybir.ActivationFunctionType.Copy,
                             bias=MAGIC, scale=qsc[:p])
        qt = qp.tile([128, SHARD], BF16, tag="q")
        nc.vector.tensor_scalar_sub(qt[:p, :], st[:p, :], MAGIC)
        for n in range(SHARD // 512):
            nc.tensor.matmul(ps_a1[:, n * 512:(n + 1) * 512], w1c[k][:p, :],
                             qt[:p, n * 512:(n + 1) * 512],
                             start=(k == 0), stop=(k == NK - 1))

    # ---------- relu(+fused sum) + local stats (max, sumsq of r) ----------
    r = hp.tile([HID, SHARD], F32, tag="r")
    s1 = sp.tile([HID, 4], F32, tag="s1")
    nc.scalar.activation(r[:], ps_a1[:], mybir.ActivationFunctionType.Relu,
                         bias=b1i[:], scale=1.0, accum_out=s1[:, 1:2])
    nc.vector.reduce_max(s1[:, 0:1], r[:], axis=mybir.AxisListType.X)
    scr = hp.tile([HID, SHARD], F32, tag="scr")
    nc.scalar.activation(scr[:], r[:], mybir.ActivationFunctionType.Square,
                         accum_out=s1[:, 2:3])

    ps3 = pss.tile([4, HID], F32, tag="psm")
    nc.tensor.transpose(ps3[:], s1[:], id_sb[:HID, :HID])
    srow = sp.tile([4, HID], F32, tag="srow")
    nc.vector.tensor_copy(srow[:], ps3[:])
    lmax = sp.tile([1, 1], F32, tag="lmax")
    nc.vector.reduce_max(lmax[:], srow[0:1, :], axis=mybir.AxisListType.X)

    # ---------- ONE AllGather: [max_r | sum_r(100) | sumsq_r(100)] ----------
    din = dcc.tile([1, 201], F32, tag="di_bn")
    dout = dcc.tile([8, 201], F32, tag="do_bn")
    nc.sync.dma_start(din[0:1, 0:1], lmax[:])
    nc.scalar.dma_start(din[0:1, 1:101], srow[1:2, :])
    nc.sync.dma_start(din[0:1, 101:201], srow[2:3, :])
    nc.gpsimd.collective_compute(
        "AllGather", mybir.AluOpType.bypass, replica_groups=rg,
        ins=[din.opt()], outs=[dout.opt()])
    g = sp.tile([8, 201], F32, tag="g_bn")
    nc.sync.dma_start(g[:], dout[:])
    # global max of r via a row view of the gathered column 0
    grow = sp.tile([1, 8], F32, tag="grow")
    nc.sync.dma_start(grow[:], dout[:, :].rearrange("a b -> b a")[0:1, :])
    gmaxr = sp.tile([1, 1], F32, tag="gmaxr")
    nc.vector.reduce_max(gmaxr[:], grow[:], axis=mybir.AxisListType.X)

    ps_s = pss.tile([1, 200], F32, tag="psm")
    nc.tensor.matmul(ps_s[:], ones8[:], g[:, 1:201], start=True, stop=True)
    tot = sp.tile([1, 200], F32, tag="tot")
    nc.vector.tensor_copy(tot[:], ps_s[:])
    psq = pss.tile([HID, 1], F32, tag="psm")
    nc.tensor.transpose(psq[:], tot[0:1, 0:100], one1[:])
    sumq = sp.tile([HID, 1], F32, tag="sumq")
    nc.vector.tensor_copy(sumq[:], psq[:])
    psq2 = pss.tile([HID, 1], F32, tag="psm")
    nc.tensor.transpose(psq2[:], tot[0:1, 100:200], one1[:])
    sumsq = sp.tile([HID, 1], F32, tag="sumsq")
    nc.vector.tensor_copy(sumsq[:], psq2[:])

    # ---------- q2 = round(r * 127/gmax_r)  (exact, bf16) ----------
    # pass split across engines: ACT handles half A's magic-add and half B's
    # subtract; DVE the converse. Both compute scale*x+bias in f32, so the
    # halves are bit-identical to the single-engine version.
    rrm = sp.tile([1, 1], F32, tag="rrm")
    nc.vector.reciprocal(rrm[:], gmaxr[:])
    qsc2 = bcast(rrm, HID, 127.0, "qsc2")
    Hq = SHARD // 2
    q2 = hp.tile([HID, SHARD], BF16, tag="q2")
    nc.scalar.activation(r[:, :Hq], r[:, :Hq],
                         mybir.ActivationFunctionType.Copy,
                         bias=MAGIC, scale=qsc2[:])
    nc.vector.tensor_scalar(r[:, Hq:], r[:, Hq:], qsc2[:], MAGIC,
                            mybir.AluOpType.mult, mybir.AluOpType.add)
    nc.vector.tensor_scalar_sub(q2[:, :Hq], r[:, :Hq], MAGIC)
    nc.scalar.activation(q2[:, Hq:], r[:, Hq:],
                         mybir.ActivationFunctionType.Copy,
                         bias=-MAGIC, scale=1.0)

    # ---------- BN affine from r-stats + folded linear2 coefficients ------
    # r is in integer units of sc_r = gmax*w1s/15;  s2 = gmaxr*sc_r/127.
    inv_n = 1.0 / float(BATCH)
    muq = sp.tile([HID, 1], F32, tag="muq")
    nc.vector.tensor_scalar_mul(muq[:], sumq[:], inv_n)      # mean_r
    msq = sp.tile([HID, 1], F32, tag="msq")
    nc.vector.tensor_scalar_mul(msq[:], sumsq[:], inv_n)
    mq2 = sp.tile([HID, 1], F32, tag="mq2")
    nc.scalar.square(mq2[:], muq[:])
    varq = sp.tile([HID, 1], F32, tag="varq")
    nc.vector.tensor_tensor(varq[:], msq[:], mq2[:], mybir.AluOpType.subtract)

    scb = bcast(gmax_sb, HID, w1s / 15.0, "scb")             # [HID,1] sc_r
    scb2 = sp.tile([HID, 1], F32, tag="scb2")
    nc.scalar.square(scb2[:], scb[:])
    pm = sp.tile([1, 1], F32, tag="pm")
    nc.vector.tensor_tensor(pm[:], gmaxr[:], gmax_sb, mybir.AluOpType.mult)
    s2c = w1s / (15.0 * 127.0)
    s2b = bcast(pm, HID, s2c, "s2b")                         # [HID,1] s2
    corrb = sp.tile([HID, 1], F32, tag="corrb")
    nc.scalar.square(corrb[:], s2b[:])
    nc.vector.tensor_scalar_mul(corrb[:], corrb[:], 1.0 / 12.0)

    var = sp.tile([HID, 1], F32, tag="var")
    nc.vector.tensor_tensor(var[:], varq[:], scb2[:], mybir.AluOpType.mult)
    nc.vector.tensor_tensor(var[:], var[:], corrb[:], mybir.AluOpType.add)
    nc.vector.tensor_scalar_add(var[:], var[:], BN_EPS)
    sd = sp.tile([HID, 1], F32, tag="sd")
    nc.scalar.sqrt(sd[:], var[:])
    isd = sp.tile([HID, 1], F32, tag="isd")
    nc.vector.reciprocal(isd[:], sd[:])
    abn = sp.tile([HID, 1], F32, tag="abn")
    nc.vector.tensor_tensor(abn[:], gam_sb, isd[:], mybir.AluOpType.mult)
    mu = sp.tile([HID, 1], F32, tag="mu")
    nc.vector.tensor_tensor(mu[:], muq[:], scb[:], mybir.AluOpType.mult)
    amu = sp.tile([HID, 1], F32, tag="amu")
    nc.vector.tensor_tensor(amu[:], abn[:], mu[:], mybir.AluOpType.mult)
    cbn = sp.tile([HID, 1], F32, tag="cbn")
    nc.vector.tensor_tensor(cbn[:], bet_sb, amu[:],
                            mybir.AluOpType.subtract)
    # w_eff = w2int * (abn*s2*w2s), split into 2 exact bf16 limbs
    abns = sp.tile([HID, 1], F32, tag="abns")
    nc.vector.tensor_scalar(abns[:], abn[:], s2b[:], w2s,
                            mybir.AluOpType.mult, mybir.AluOpType.mult)
    ah = split2(abns, HID, "ah")
    weffs = []
    for j in range(2):
        wj = sp.tile([HID, OUT], BF16, tag=f"weff{j}")
        nc.vector.tensor_scalar_mul(wj[:], w2t_sb[:], ah[j][1][:])
        weffs.append(wj)
    # zbias[1,2] = w2s*(cbn @ w2int) + b2i*(w2s*s2); cbn split for exactness
    ch = split2(cbn, HID, "ch")
    ps_zb = pss.tile([1, OUT], F32, tag="psm")
    for j in range(2):
        nc.tensor.matmul(ps_zb[:], ch[j][0][:], w2t_sb[:],
                         start=(j == 0), stop=(j == 1))
    zb1 = sp.tile([1, OUT], F32, tag="zb1")
    nc.vector.tensor_scalar_mul(zb1[:], ps_zb[:], w2s)
    s2_sc = sp.tile([1, 1], F32, tag="s2sc")
    nc.vector.tensor_scalar_mul(s2_sc[:], pm[:], s2c)        # [1,1] s2
    rs2 = sp.tile([1, 1], F32, tag="rs2")
    nc.vector.reciprocal(rs2[:], s2_sc[:])
    b2sc = sp.tile([1, 1], F32, tag="b2sc")
    nc.vector.tensor_scalar_mul(b2sc[:], rs2[:], 1.0 / w2s)  # 1/(w2s*s2)
    t3 = sp.tile([1, OUT], F32, tag="t3")
    nc.scalar.activation(t3[:], b2_sb, mybir.ActivationFunctionType.Copy,
                         bias=MAGIC, scale=b2sc[:])
    b2i = sp.tile([1, OUT], F32, tag="b2i")
    nc.vector.tensor_scalar(b2i[:], t3[:], MAGIC, 1.0,
                            mybir.AluOpType.subtract, mybir.AluOpType.min)
    nc.vector.tensor_scalar_max(b2i[:], b2i[:], -2.0)
    b2is = sp.tile([1, OUT], F32, tag="b2is")
    nc.vector.tensor_scalar(b2is[:], b2i[:], s2_sc[:], w2s,
                            mybir.AluOpType.mult, mybir.AluOpType.mult)
    zbias = sp.tile([1, OUT], F32, tag="zbias")
    nc.vector.tensor_tensor(zbias[:], zb1[:], b2is[:], mybir.AluOpType.add)
    ps_zbt = pss.tile([OUT, 1], F32, tag="psm")
    nc.tensor.transpose(ps_zbt[:], zbias[:], one1[:])
    zb2 = sp.tile([OUT, 1], F32, tag="zb2")
    nc.vector.tensor_copy(zb2[:], ps_zbt[:])

    # ---------- GEMM2 (2 exact bf16 limbs) + relu + quant ----------
    ps_z = psb.tile([OUT, SHARD], F32, tag="big")
    for n in range(SHARD // 512):
        for j in range(2):
            nc.tensor.matmul(ps_z[:, n * 512:(n + 1) * 512], weffs[j][:],
                             q2[:, n * 512:(n + 1) * 512],
                             start=(j == 0), stop=(j == 1))
    zr = zp.tile([OUT, SHARD], F32, tag="zr")
    nc.scalar.activation(zr[:], ps_z[:], mybir.ActivationFunctionType.Relu,
                         bias=zb2[:], scale=1.0)
    zm1 = sp.tile([OUT, 1], F32, tag="zm1")
    nc.vector.reduce_max(zm1[:], zr[:], axis=mybir.AxisListType.X)

    # ---------- AllGather #2: per-core [2,1] maxes -> [16,1] column -------
    din2 = dcc.tile([OUT, 1], F32, tag="di_z")
    dout2 = dcc.tile([8 * OUT, 1], F32, tag="do_z")
    nc.sync.dma_start(din2[:], zm1[:])
    nc.gpsimd.collective_compute(
        "AllGather", mybir.AluOpType.bypass, replica_groups=rg,
        ins=[din2.opt()], outs=[dout2.opt()])
    g4 = sp.tile([1, 8 * OUT], F32, tag="g_z")
    nc.sync.dma_start(g4[:], dout2[:, :].rearrange("a b -> b a"))
    gmaxz = sp.tile([1, 1], F32, tag="gmaxz")
    nc.vector.reduce_max(gmaxz[:], g4[:], axis=mybir.AxisListType.X)

    rmz = sp.tile([1, 1], F32, tag="rmz")
    nc.vector.reciprocal(rmz[:], gmaxz[:])
    qsc3 = bcast(rmz, OUT, 127.0, "qsc3")          # [2,1] 127/maxz
    s3b = bcast(gmaxz, OUT, 1.0 / 127.0, "s3b")    # [2,1] s3
    nbias = sp.tile([OUT, 1], F32, tag="nbias")    # -MAGIC*s3
    nc.vector.tensor_scalar_mul(nbias[:], s3b[:], -MAGIC)
    # final pass split across engines: ACT does half A then half B's rescale,
    # DVE does half B then half A's rescale. (values >= 0 so Relu == Copy)
    H = SHARD // 2
    t5 = zp.tile([OUT, SHARD], F32, tag="t5")
    osb = zp.tile([OUT, SHARD], F32, tag="osb")
    nc.scalar.activation(t5[:, :H], zr[:, :H],
                         mybir.ActivationFunctionType.Copy,
                         bias=MAGIC, scale=qsc3[:])
    nc.vector.tensor_scalar(t5[:, H:], zr[:, H:], qsc3[:], MAGIC,
                            mybir.AluOpType.mult, mybir.AluOpType.add)
    nc.vector.tensor_scalar(osb[:, :H], t5[:, :H], MAGIC, s3b[:],
                            mybir.AluOpType.subtract, mybir.AluOpType.mult)
    nc.scalar.activation(osb[:, H:], t5[:, H:],
                         mybir.ActivationFunctionType.Relu,
                         bias=nbias[:], scale=s3b[:])
    nc.sync.dma_start(out[:, :], osb[:])


def _prep(sig, W1, b1, W2, b2, gamma, beta):
    sig = np.asarray(sig, dtype=np.float32)
    W1 = np.asarray(W1, dtype=np.float32)
    W2 = np.asarray(W2, dtype=np.float32)
    w1s = float(np.max(np.abs(W1)))
    w2s = float(np.max(np.abs(W2)))
    gmax = float(np.max(np.abs(sig)))
    w1i = np.clip(np.round(W1 / w1s), -2, 1).astype(np.float32)
    w2i = np.clip(np.round(W2 / w2s), -2, 1).astype(np.float32)
    w1t = np.zeros((KP, HID), dtype=ml_dtypes.bfloat16)
    w1t[:D_IN, :] = w1i.T.astype(ml_dtypes.bfloat16)
    w2t = np.ascontiguousarray(w2i.T).astype(ml_dtypes.bfloat16)
    pvec = np.zeros((HID, 8), dtype=np.float32)
    pvec[:, 0] = np.asarray(b1, np.float32)
    pvec[:, 1] = np.asarray(gamma, np.float32)
    pvec[:, 2] = np.asarray(beta, np.float32)
    pvec[0, 3] = gmax
    pvec[0, 4:6] = np.asarray(b2, np.float32)
    # feature-major shards: [8, 2000, 2048], one contiguous pass
    sigT8 = np.ascontiguousarray(
        sig.reshape(NCORES, SHARD, D_IN).transpose(0, 2, 1))
    com = {
        "w1t": w1t,
        "w2t": w2t,
        "pvec": pvec,
        "ident": np.eye(128, dtype=np.float32),
    }
    in_maps = []
    for c in range(NCORES):
        m = dict(com)
        m["sigT"] = sigT8[c]
        in_maps.append(m)
    return w1s, w2s, in_maps


def kernel(sig, W1, b1, W2, b2, gamma, beta):
    w1s, w2s, in_maps = _prep(sig, W1, b1, W2, b2, gamma, beta)
    key = (round(w1s, 9), round(w2s, 9))
    if key not in _CACHE:
        _CACHE[key] = _build(w1s, w2s)
    nc = _CACHE[key]
    trace = os.environ.get("BASS_TRACE") == "1"
    try:
        res = bass_utils.run_bass_kernel_spmd(
            nc, in_maps, core_ids=list(range(NCORES)), trace=trace)
    except ModuleNotFoundError:
        os.environ["BASS_NEVER_TRACE"] = "1"
        res = bass_utils.run_bass_kernel_spmd(
            nc, in_maps, core_ids=list(range(NCORES)), trace=False)
    kernel.last_results = res
    return np.concatenate(
        [np.ascontiguousarray(r["out"].T) for r in res.results], axis=0)
